# revision 18
# baseline (speedup 1.0000x reference)
"""Trainium2 Bass kernel for the AttentionHook module.

Math (per batch b, N = H*W = 4096):
    f = wq @ x   [N];   g = wk @ x   [N];   h = wv @ x   [C, N]
    scores[i, j] = f[i] * g[j]      (rank-1 outer product!)
    beta = softmax(scores, axis=0)  (normalize over i, per column j)
    o = (1-gamma) * h @ beta + gamma * x

Restructuring: the [N, N] score matrix is never materialized in HBM.
    o[c, m] = sum_n h[c, n] * E[n, m] / Z[m],  E = exp(f_n * g_m),
    Z[m] = sum_n E[n, m].
Per core (one batch per core, 8 cores):
  - E tiles [128n, 1024m] are each ONE ScalarE op:
    activation(Exp, in_=g_bcast, scale=fT chunk) == exp(f_p * g_m).
  - TensorE accumulates outT[m, c'] = sum_n E[n, m] * hT_aug[n, c'],
    hT_aug = [h^T | ones] in bf16 -> column C is Z: the softmax
    normalizer falls out of the same matmul chain (FWL weight loads).
  - VectorE multiplies by 1/Z per-partition (m) and streams out o^T.
Precision: x is shipped as a bf16x2 split (x = xh + xl exactly to
~2^-17), so the f/g projections (exponent-sensitive!) are computed by
THREE bf16 matmul terms (xh*wh + xl*wh + xh*wl) at bf16 speed but
near-fp32 accuracy; h needs only plain bf16 accuracy (xh*wh).
The host transposes o^T back and applies the (trivial) gamma blend.
"""

import numpy as np
from contextlib import ExitStack

B, C, HH, WW = 8, 256, 64, 64
N = HH * WW            # 4096
P = 128
NCH = N // P           # 32 n-chunks
CCH = C // P           # 2 c-chunks
HWID = C + 1           # 257: h columns + ones column (Z)
RWID = C + 3           # 259: stage-C psum: h | f_hh | f_hl | f_lh
MG = 8                 # m-chunks per PSUM group (8 banks)
GW = MG * P            # 1024: m-group width (ACT op width)
NGRP = N // GW         # 4 m-groups
GB = 512               # stage-B m-chunk width (full PSUM bank)

_CACHE = {}


def _build():
    import concourse.tile as tile
    from concourse import bacc, mybir

    f32 = mybir.dt.float32
    bf16 = mybir.dt.bfloat16
    Exp = mybir.ActivationFunctionType.Exp

    nc = bacc.Bacc("TRN2", target_bir_lowering=False, debug=False)
    xh_d = nc.dram_tensor("xh", [C, N], bf16, kind="ExternalInput").ap()
    xl_d = nc.dram_tensor("xl", [C, N], bf16, kind="ExternalInput").ap()
    # [wv^T | wq_hi^T | wq_lo^T | wq_hi^T | wk_hi^T rep | wk_lo^T rep]
    WA = (C + 2) + 1 + P + P  # 515
    wall_d = nc.dram_tensor("w_all", [C, WA], bf16, kind="ExternalInput").ap()
    o_d = nc.dram_tensor("o", [N, C], f32, kind="ExternalOutput").ap()

    with tile.TileContext(nc) as tc, ExitStack() as ctx:
        cpool = ctx.enter_context(tc.tile_pool(name="cpool", bufs=1))

        xh_sb = [cpool.tile([P, N], bf16, tag=f"xh{c}", name=f"xh_sb{c}")
                 for c in range(CCH)]
        xl_sb = [cpool.tile([P, N], bf16, tag=f"xl{c}", name=f"xl_sb{c}")
                 for c in range(CCH)]
        wall_sb = [cpool.tile([P, WA], bf16, tag=f"wall{c}", name=f"wall_sb{c}")
                   for c in range(CCH)]
        wvq_sb = [t[:, 0:C + 2] for t in wall_sb]
        wqh_sb = [t[:, C + 2:C + 3] for t in wall_sb]
        wkh_sb = [t[:, C + 3:C + 3 + P] for t in wall_sb]
        wkl_sb = [t[:, C + 3 + P:C + 3 + 2 * P] for t in wall_sb]
        g_sb = cpool.tile([P, N], f32, tag="g")          # g on all partitions
        ht_sb = cpool.tile([P, NCH * HWID], bf16, tag="ht")  # hT_aug per n-chunk
        ft_sb = cpool.tile([P, NCH], f32, tag="ft")      # f^T, col n = chunk n
        ftp_sb = cpool.tile([P, 2 * NCH], f32, tag="ftp")  # f^T partial terms

        # DMA issue costs ~0.6us of sequencer time per dma_start: use few,
        # large transfers and spread issue across idle engine queues. The
        # first 1024 columns land early so stage B/C start immediately.
        s0, s1 = slice(0, 2 * GB), slice(2 * GB, N)
        nc.sync.dma_start(wall_sb[0][:], wall_d[0:P, :])
        nc.scalar.dma_start(wall_sb[1][:], wall_d[P:C, :])
        nc.sync.dma_start(xh_sb[0][:, s0], xh_d[0:P, s0])
        nc.scalar.dma_start(xh_sb[1][:, s0], xh_d[P:C, s0])
        nc.gpsimd.dma_start(xl_sb[0][:, s0], xl_d[0:P, s0])
        nc.gpsimd.dma_start(xl_sb[1][:, s0], xl_d[P:C, s0])
        for c in range(CCH):
            nc.sync.dma_start(xh_sb[c][:, s1], xh_d[c * P:(c + 1) * P, s1])
            nc.gpsimd.dma_start(xl_sb[c][:, s1], xl_d[c * P:(c + 1) * P, s1])

        bc_ctx = ctx.enter_context(ExitStack())
        psum_g = bc_ctx.enter_context(tc.tile_pool(name="psum_g", bufs=2, space="PSUM"))
        psum_h = bc_ctx.enter_context(tc.tile_pool(name="psum_h", bufs=4, space="PSUM"))
        terms = [(wkh_sb, xh_sb), (wkl_sb, xh_sb), (wkh_sb, xl_sb)]

        def stage_b(j):
            # g_bcast[p, j*GB:(j+1)*GB] = g[m]: three bf16x2 matmul terms
            pg = psum_g.tile([P, GB], f32, tag="pg", name=f"pg{j}")
            for t, (wt, xt) in enumerate(terms):
                for c in range(CCH):
                    nc.tensor.matmul(
                        pg[:], wt[c][:], xt[c][:, j * GB:(j + 1) * GB],
                        start=(t == 0 and c == 0),
                        stop=(t == len(terms) - 1 and c == CCH - 1),
                    )
            nc.vector.tensor_copy(g_sb[:, j * GB:(j + 1) * GB], pg[:])

        def stage_c(n):
            # hT_aug (bf16) + near-fp32 fT: psum cols 0:C = h^T, cols
            # C:C+2 get xh*[wq_hi | wq_lo], and xl*wq_hi adds onto col C.
            ph = psum_h.tile([P, RWID], f32, tag="ph", name=f"ph{n}")
            for c in range(CCH):
                nc.tensor.matmul(
                    ph[:, 0:C + 2], xh_sb[c][:, n * P:(n + 1) * P],
                    wvq_sb[c][:], start=(c == 0), stop=False,
                    skip_group_check=True,
                )
            for c in range(CCH):
                nc.tensor.matmul(
                    ph[:, C:C + 1], xl_sb[c][:, n * P:(n + 1) * P],
                    wqh_sb[c][:], start=False, stop=(c == CCH - 1),
                    skip_group_check=True,
                )
            nc.vector.tensor_copy(ht_sb[:, n * HWID:n * HWID + C], ph[:, 0:C])
            nc.vector.tensor_copy(ftp_sb[:, 2 * n:2 * n + 2], ph[:, C:C + 2])
            nc.vector.tensor_add(ft_sb[:, n:n + 1], ftp_sb[:, 2 * n:2 * n + 1],
                                 ftp_sb[:, 2 * n + 1:2 * n + 2])
            nc.gpsimd.memset(ht_sb[:, n * HWID + C:n * HWID + C + 1], 1.0)

        # Interleave: group-0's g columns first, then fT/hT chunks stream
        # in while the remaining g columns fill in.
        stage_b(0)
        stage_b(1)
        for n in range(8):
            stage_c(n)
        for j in range(2, N // GB):
            stage_b(j)
            for n in range(4 * j, 4 * j + 4):
                stage_c(n)
        bc_ctx.close()

        # main: for each m-group, accumulate outT[m, c'] over all n-chunks
        with tc.tile_pool(name="epool", bufs=24) as epool, \
             tc.tile_pool(name="psum_o", bufs=MG, space="PSUM") as psum_o, \
             tc.tile_pool(name="outp", bufs=8) as outp, \
             tc.tile_pool(name="rzp", bufs=8) as rzp:
            for g in range(NGRP):
                po = [psum_o.tile([P, HWID], f32, tag="po", name=f"po_{g}_{i}")
                      for i in range(MG)]
                for n in range(NCH):
                    et = epool.tile([P, GW], bf16, tag="et", name=f"et_{g}_{n}")
                    nc.scalar.activation(
                        et[:], g_sb[:, g * GW:(g + 1) * GW], Exp,
                        scale=ft_sb[:, n:n + 1],
                    )
                    for mc in range(MG):
                        nc.tensor.matmul(
                            po[mc][:], et[:, mc * P:(mc + 1) * P],
                            ht_sb[:, n * HWID:(n + 1) * HWID],
                            start=(n == 0), stop=(n == NCH - 1),
                        )
                dma_eng = [nc.sync, nc.gpsimd]
                for mc in range(MG):
                    rz = rzp.tile([P, 1], f32, tag="rz", name=f"rz_{g}_{mc}")
                    nc.vector.reciprocal(rz[:], po[mc][:, C:C + 1])
                    ot = outp.tile([P, C], f32, tag="ot", name=f"ot_{g}_{mc}")
                    nc.vector.tensor_scalar_mul(ot[:], po[mc][:, 0:C], rz[:])
                    m0 = g * GW + mc * P
                    dma_eng[mc % 2].dma_start(o_d[m0:m0 + P, :], ot[:])

    nc.compile()
    return nc


def _get_nc():
    if "nc" not in _CACHE:
        _CACHE["nc"] = _build()
    return _CACHE["nc"]


def _bf16_split(a):
    import ml_dtypes
    hi = a.astype(ml_dtypes.bfloat16)
    lo = (a - hi.astype(np.float32)).astype(ml_dtypes.bfloat16)
    return hi, lo


def make_in_maps(x, wq, wk, wv):
    import ml_dtypes
    bf = ml_dtypes.bfloat16
    xf = np.ascontiguousarray(x, dtype=np.float32).reshape(B, C, N)
    wq = np.asarray(wq, dtype=np.float32).reshape(C)
    wk = np.asarray(wk, dtype=np.float32).reshape(C)
    wv = np.asarray(wv, dtype=np.float32)

    wqh, wql = _bf16_split(wq)
    wkh, wkl = _bf16_split(wk)
    # [wv^T | wq_hi | wq_lo | wq_hi | wk_hi rep | wk_lo rep] -> [C, 515]
    w_all = np.concatenate([
        wv.T.astype(bf),
        wqh.reshape(C, 1), wql.reshape(C, 1), wqh.reshape(C, 1),
        np.repeat(wkh.reshape(C, 1), P, axis=1),
        np.repeat(wkl.reshape(C, 1), P, axis=1),
    ], axis=1)
    w_all = np.ascontiguousarray(w_all)

    in_maps = []
    for b in range(B):
        xh, xl = _bf16_split(xf[b])
        in_maps.append({
            "xh": np.ascontiguousarray(xh),
            "xl": np.ascontiguousarray(xl),
            "w_all": w_all,
        })
    return in_maps, xf


def kernel(x, wq, wk, wv, gamma):
    from concourse.bass_utils import run_bass_kernel_spmd

    in_maps, xf = make_in_maps(x, wq, wk, wv)
    nc = _get_nc()
    res = run_bass_kernel_spmd(nc, in_maps, core_ids=list(range(B)))

    g0 = float(np.asarray(gamma, dtype=np.float32).reshape(-1)[0])
    out = np.empty((B, C, HH, WW), dtype=np.float32)
    for b in range(B):
        o = res.results[b]["o"].T  # [C, N]
        if g0 != 0.0:
            o = (1.0 - g0) * o + g0 * xf[b]
        out[b] = o.reshape(C, HH, WW)
    return out
